# revision 3
# baseline (speedup 1.0000x reference)
"""Trainium2 Bass kernel for nn_MultiHeadAttention_39582418600023.

Model (reference bug preserved: Q = K = V = x @ W_Q):
  qkv = x @ W_Q; q,k,v = heads(qkv)
  out = softmax(causal(q k^T) / sqrt(dh)) v  ->  ctx @ W_out + b_out

Sharding (8 cores): data-parallel over batch (4) x tensor-parallel over
head groups (2).  Core c handles batch c//2, heads (c%2)*8 .. +8
(W_Q column-parallel, W_out row-parallel); host sums the two partial
out-projections per batch and adds the bias.

Per-core device kernel (all matmuls in float32r - full PE rate, ~1e-4 rel):
  1. qkvT[e,t] = W_Qc^T @ x^T     (scores operands, transposed layout)
     qkv [t,e] -> VA[t, h, 0:64]=V, VA[t, h, 64]=1.0  (ones-augmented V)
  2. per head h, key block kb: ST[k,q] = K_kb^T Q  (only q >= kb*128),
     PT = exp(ST/8)  (no max subtraction needed: scores <= ~20),
     upper-tri 0/1 mask on the diagonal 128x128 block (gpsimd).
  3. per query chunk qc: C[0:65,q] = sum_kb VA_kb^T @ PT_kb
     -> rows 0:64 = unnormalized ctxT, row 64 = softmax denominator.
     recip(denom) -> broadcast to 128 partitions via K=1 matmul ->
     ctxT = ctxT * recip  (normalization).
  4. out_partial = ctxT^T @ W_outc  (row-parallel partial, host reduces).
"""
import os
import sys

sys.path.insert(0, "/opt/trn_rl_repo")
os.environ.setdefault("MYCRO_LOCAL_CACHE", "1")

import numpy as np

B, S, D = 4, 2048, 1024
NH, DH = 16, 64
EH = 512          # e-columns per core (8 local heads)
NHL = 8           # local heads
N_CORES = 8

_CACHE = {}


def _build():
    import concourse.mybir as mybir
    import concourse.tile as tile
    from concourse import bacc
    from concourse.masks import make_upper_triangular

    F32 = mybir.dt.float32
    F32R = mybir.dt.float32r
    EXP = mybir.ActivationFunctionType.Exp

    nc = bacc.Bacc(None, target_bir_lowering=False, debug=True)
    with tile.TileContext(nc) as tc:
        with tc.tile_pool(name="dram", bufs=1, space="DRAM") as dram:
            xT = dram.tile([D, S], F32, kind="ExternalInput")      # x[b].T
            wq = dram.tile([D, EH], F32, kind="ExternalInput")     # W_Q cols
            wo = dram.tile([EH, D], F32, kind="ExternalInput")     # W_out rows
            outp = dram.tile([S, D], F32, kind="ExternalOutput")   # partial out

            with tc.tile_pool(name="persist", bufs=1) as pp:
                # qkvT: [e-block 128, eb, t]  e = eb*128+p
                QKVT = pp.tile([128, 4, S], F32R)
                # ones-augmented V: [t%128, tb, h, 0:64]=V, [..,64]=1
                VA = pp.tile([128, 16, NHL, DH + 1], F32R)
                # normalized ctxT, same layout as QKVT
                CTXT = pp.tile([128, 4, S], F32R)
                MASK = pp.tile([128, 128], F32)   # 1 on i<=j else 0
                make_upper_triangular(nc, MASK[:], val=1.0, diag=True)
                nc.vector.memset(VA[:, :, :, DH : DH + 1].bitcast(F32), 1.0)
                # broadcast lhsT: row 64 of a [65,128] tile = 1.0
                OBC = pp.tile([65, 128], F32R)
                nc.vector.memset(OBC[64:65, :].bitcast(F32), 1.0)
                # denominator staging (partition 64 holds data)
                DN = pp.tile([65, 512], F32)
                REC = pp.tile([65, 512], F32R)

                # ---------------- phase 1: projections ----------------
                with tc.tile_pool(name="px", bufs=1) as px, \
                     tc.tile_pool(name="pj", bufs=4, space="PSUM") as pj:
                    XT = px.tile([128, 8, S], F32R)
                    WQ = px.tile([128, 8, EH], F32R)
                    for kc in range(8):
                        nc.sync.dma_start(
                            out=XT[:, kc, :],
                            in_=xT[kc * 128 : (kc + 1) * 128, :].bitcast(F32R))
                        nc.sync.dma_start(
                            out=WQ[:, kc, :],
                            in_=wq[kc * 128 : (kc + 1) * 128, :].bitcast(F32R))
                    # qkvT = wq^T @ xT
                    for eb in range(4):
                        for tn in range(4):
                            ps = pj.tile([128, 512], F32, tag="pj")
                            for kc in range(8):
                                nc.tensor.matmul(
                                    ps[:],
                                    WQ[:, kc, eb * 128 : (eb + 1) * 128],
                                    XT[:, kc, tn * 512 : (tn + 1) * 512],
                                    start=(kc == 0), stop=(kc == 7))
                            nc.scalar.copy(
                                QKVT[:, eb, tn * 512 : (tn + 1) * 512], ps[:])
                    # qkv = xT^T @ wq -> VA
                    for tb in range(16):
                        ps = pj.tile([128, 512], F32, tag="pj")
                        for kc in range(8):
                            nc.tensor.matmul(
                                ps[:],
                                XT[:, kc, tb * 128 : (tb + 1) * 128],
                                WQ[:, kc, :],
                                start=(kc == 0), stop=(kc == 7))
                        nc.vector.tensor_copy(
                            VA[:, tb, :, 0:DH],
                            ps[:].rearrange("p (h d) -> p h d", h=NHL))

                # ---------------- phase 2: attention ----------------
                with tc.tile_pool(name="pt", bufs=1) as ptp, \
                     tc.tile_pool(name="psc", bufs=1, space="PSUM") as psc, \
                     tc.tile_pool(name="ppv", bufs=2, space="PSUM") as ppv, \
                     tc.tile_pool(name="pbc", bufs=1, space="PSUM") as pbc:
                    for h in range(NHL):
                        half, jb = h % 2, h // 2
                        qh = QKVT[half * 64 : half * 64 + 64, jb, :]  # [64,S]
                        pts = []
                        for kb in range(16):
                            L = S - kb * 128
                            sc = psc.tile([128, S], F32, tag="sc")
                            for lo in range(0, L, 512):
                                n = min(512, L - lo)
                                nc.tensor.matmul(
                                    sc[:, lo : lo + n],
                                    qh[:, kb * 128 : (kb + 1) * 128],
                                    qh[:, kb * 128 + lo : kb * 128 + lo + n],
                                    start=True, stop=True)
                            pt = ptp.tile([128, L], F32R, tag=f"pt{kb}")
                            nc.scalar.activation(
                                pt[:], sc[:, 0:L], EXP, scale=0.125)
                            nc.vector.tensor_mul(
                                pt[:, 0:128], pt[:, 0:128], MASK[:])
                            pts.append(pt)
                        for qc in range(4):
                            qs = qc * 512
                            nkb = 4 * qc + 4
                            C = ppv.tile([65, 512], F32, tag="pv")
                            for kb in range(nkb):
                                po = max(0, kb * 128 - qs)
                                ls = qs + po - kb * 128
                                w = 512 - po
                                nc.tensor.matmul(
                                    C[:, po:512],
                                    VA[:, kb, h, :],
                                    pts[kb][:, ls : ls + w],
                                    start=(kb == 0), stop=(kb == nkb - 1))
                            nc.vector.tensor_copy(DN[64:65, :], C[64:65, :])
                            with nc.allow_low_precision(
                                    reason="f32r recip, 1e-4 ok"):
                                nc.vector.reciprocal(
                                    REC[64:65, :], DN[64:65, :])
                            BC = pbc.tile([128, 512], F32, tag="bc")
                            nc.tensor.matmul(
                                BC[:], OBC[64:65, :], REC[64:65, :],
                                start=True, stop=True)
                            dst = CTXT[half * 64 : half * 64 + 64, jb,
                                       qs : qs + 512]
                            nc.vector.tensor_copy(dst, C[0:64, :])
                            nc.vector.tensor_mul(dst, dst, BC[0:64, :])

                # ---------------- phase 3: out projection ----------------
                with tc.tile_pool(name="po", bufs=3) as po, \
                     tc.tile_pool(name="pop", bufs=4, space="PSUM") as pop:
                    WO = po.tile([128, 4, D], F32R, tag="wo")
                    for eb in range(4):
                        nc.sync.dma_start(
                            out=WO[:, eb, :],
                            in_=wo[eb * 128 : (eb + 1) * 128, :].bitcast(F32R))
                    for tb in range(16):
                        for nn in range(2):
                            ps = pop.tile([128, 512], F32, tag="pop")
                            for eb in range(4):
                                nc.tensor.matmul(
                                    ps[:],
                                    CTXT[:, eb, tb * 128 : (tb + 1) * 128],
                                    WO[:, eb, nn * 512 : (nn + 1) * 512],
                                    start=(eb == 0), stop=(eb == 3))
                            ob = po.tile([128, 512], F32, tag="ob")
                            nc.vector.tensor_copy(ob[:], ps[:])
                            nc.sync.dma_start(
                                out=outp[tb * 128 : (tb + 1) * 128,
                                         nn * 512 : (nn + 1) * 512],
                                in_=ob[:])
    nc.compile()
    return nc, {"xT": xT.name, "wq": wq.name, "wo": wo.name,
                "outp": outp.name}


def _get():
    if "nc" not in _CACHE:
        _CACHE["nc"], _CACHE["names"] = _build()
    return _CACHE["nc"], _CACHE["names"]


def _run(x, W_Q, W_out, trace=False):
    from concourse.bass_utils import run_bass_kernel_spmd

    nc, nm = _get()
    in_maps = []
    for c in range(N_CORES):
        b, hg = c // 2, c % 2
        in_maps.append({
            nm["xT"]: np.ascontiguousarray(x[b].T.astype(np.float32)),
            nm["wq"]: np.ascontiguousarray(
                W_Q[:, hg * EH : (hg + 1) * EH].astype(np.float32)),
            nm["wo"]: np.ascontiguousarray(
                W_out[hg * EH : (hg + 1) * EH, :].astype(np.float32)),
        })
    return run_bass_kernel_spmd(
        nc, in_maps, list(range(N_CORES)), trace=trace), nm


def kernel(x, W_Q, W_out, b_out):
    res, nm = _run(np.asarray(x), np.asarray(W_Q), np.asarray(W_out))
    bo = np.asarray(b_out, dtype=np.float32)
    out = np.empty((B, S, D), np.float32)
    for b in range(B):
        out[b] = (res.results[2 * b][nm["outp"]]
                  + res.results[2 * b + 1][nm["outp"]] + bo)
    return out


# revision 6
# speedup vs baseline: 1.5857x; 1.5857x over previous
"""Trainium2 Bass kernel for nn_MultiHeadAttention_39582418600023.

Model (reference bug preserved: Q = K = V = x @ W_Q):
  qkv = x @ W_Q; q,k,v = heads(qkv)
  out = softmax(causal(q k^T) / sqrt(dh)) v  ->  ctx @ W_out + b_out

Sharding (8 cores): data-parallel over batch (4) x tensor-parallel over
head groups (2).  Core c handles batch c//2, heads (c%2)*8 .. +8
(W_Q column-parallel, W_out row-parallel); host sums the two partial
out-projections per batch and adds the bias.

Per-core device kernel:
  dtypes: projections + out-proj in float32r (full PE rate at K=128,
  ~1e-4 rel err); attention matmuls in bf16 (score/prob quantization
  errors average out across the softmax dot products, ~1e-3 total).

  1. qkvT[e,t] (bf16, scores operands) and qkv->VA (bf16, ones-augmented
     V: VA[t,h,64]=1) via two fp32r projections from x^T.
  2. Head-pair loop (pairs share a 128-partition block of qkvT):
     scores ST[k,q] for both heads land in one [128,1024] PSUM tile via
     two concurrent row-tiled matmuls (base partitions 0/64); one
     exp-ACTIVATE per tile writes both heads' probs PT (bf16).
     Upper-tri 0/1 mask on the diagonal 128x128 block.
  3. PV: C[0:65] = sum_kb VA_kb^T @ PT_kb -> rows 0:64 unnormalized
     ctxT, row 64 softmax denominator.  ctxT evicted unnormalized
     (fp32r); denominators staged across 128 partitions.
  4. Post-attention: one reciprocal [128,128], denominator rows
     rebuilt at partitions {0,32,64,96} via DMA, broadcast to all
     partitions with K=1 matmuls, ctxT *= recip (DVE).
  5. out_partial = ctxT^T @ W_outc (fp32r); host reduces + bias.
"""
import os
import sys

sys.path.insert(0, "/opt/trn_rl_repo")
os.environ.setdefault("MYCRO_LOCAL_CACHE", "1")

import numpy as np

B, S, D = 4, 2048, 1024
NH, DH = 16, 64
EH = 512          # e-columns per core (8 local heads)
NHL = 8           # local heads
N_CORES = 8

_CACHE = {}


def _build():
    import concourse.mybir as mybir
    import concourse.tile as tile
    from concourse import bacc
    from concourse.masks import make_upper_triangular

    F32 = mybir.dt.float32
    F32R = mybir.dt.float32r
    BF16 = mybir.dt.bfloat16
    EXP = mybir.ActivationFunctionType.Exp

    nc = bacc.Bacc(None, target_bir_lowering=False, debug=True)
    with tile.TileContext(nc) as tc:
        with tc.tile_pool(name="dram", bufs=1, space="DRAM") as dram:
            xT = dram.tile([D, S], F32, kind="ExternalInput")      # x[b].T
            wq = dram.tile([D, EH], F32, kind="ExternalInput")     # W_Q cols
            wo = dram.tile([EH, D], F32, kind="ExternalInput")     # W_out rows
            outp = dram.tile([S, D], F32, kind="ExternalOutput")   # partial out

            with tc.tile_pool(name="persist", bufs=1) as pp:
                # qkvT: [e-block 128, eb, t], bf16 (scores operands)
                QKVT = pp.tile([128, 4, S], BF16)
                # ones-augmented V (bf16): [t%128, tb, h, 0:64]=V, [..,64]=1
                VA = pp.tile([128, 16, NHL, DH + 1], BF16)
                # unnormalized ctxT (fp32r), same layout as QKVT
                CTXT = pp.tile([128, 4, S], F32R)
                MASK = pp.tile([128, 128], F32)   # 1 on i<=j else 0
                make_upper_triangular(nc, MASK[:], val=1.0, diag=True)
                nc.vector.memset(VA[:, :, :, DH : DH + 1], 1.0)
                # denominator staging: (h,qc) pair i -> partitions 4i:4i+4
                DROW = pp.tile([65, 512], F32)
                DSTACK = pp.tile([128, 128], F32)
                RSTACK = pp.tile([128, 128], F32R)
                RROWS = pp.tile([128, 8, 512], F32R)
                ONES_F = pp.tile([128, 128], F32)
                ONES_B = pp.tile([128, 128], F32R)
                nc.vector.memset(ONES_F[:], 1.0)
                nc.vector.tensor_copy(ONES_B[:], ONES_F[:])

                # ---------------- phase 1: projections ----------------
                with tc.tile_pool(name="px", bufs=1) as px, \
                     tc.tile_pool(name="pj", bufs=4, space="PSUM") as pj:
                    XT = px.tile([128, 8, S], F32R)
                    WQ = px.tile([128, 8, EH], F32R)
                    for kc in range(8):
                        nc.sync.dma_start(
                            out=XT[:, kc, :],
                            in_=xT[kc * 128 : (kc + 1) * 128, :].bitcast(F32R))
                        nc.sync.dma_start(
                            out=WQ[:, kc, :],
                            in_=wq[kc * 128 : (kc + 1) * 128, :].bitcast(F32R))
                    # qkvT = wq^T @ xT  (evict on ACT -> bf16)
                    for eb in range(4):
                        for tn in range(4):
                            ps = pj.tile([128, 512], F32, tag="pj")
                            for kc in range(8):
                                nc.tensor.matmul(
                                    ps[:],
                                    WQ[:, kc, eb * 128 : (eb + 1) * 128],
                                    XT[:, kc, tn * 512 : (tn + 1) * 512],
                                    start=(kc == 0), stop=(kc == 7))
                            nc.scalar.copy(
                                QKVT[:, eb, tn * 512 : (tn + 1) * 512], ps[:])
                    # qkv = xT^T @ wq -> VA (evict on DVE -> bf16)
                    for tb in range(16):
                        ps = pj.tile([128, 512], F32, tag="pj")
                        for kc in range(8):
                            nc.tensor.matmul(
                                ps[:],
                                XT[:, kc, tb * 128 : (tb + 1) * 128],
                                WQ[:, kc, :],
                                start=(kc == 0), stop=(kc == 7))
                        nc.vector.tensor_copy(
                            VA[:, tb, :, 0:DH],
                            ps[:].rearrange("p (h d) -> p h d", h=NHL))

                # ---------------- phase 2: attention (head pairs) -----
                with tc.tile_pool(name="pt", bufs=1) as ptp, \
                     tc.tile_pool(name="psc", bufs=3, space="PSUM") as psc, \
                     tc.tile_pool(name="ppv", bufs=2, space="PSUM") as ppv:
                    for jb in range(4):          # head pair block
                        qA = QKVT[0:64, jb, :]   # head 2jb   [64, S]
                        qB = QKVT[64:128, jb, :]  # head 2jb+1
                        pts = []
                        for kb in range(16):
                            L = S - kb * 128
                            pt = ptp.tile([128, 2, L], BF16, tag=f"pt{kb}")
                            for lo in range(0, L, 512):
                                n = min(512, L - lo)
                                sc = psc.tile([128, 1024], F32, tag="sc")
                                k0 = kb * 128
                                nc.tensor.matmul(
                                    sc[:, 0:n],
                                    qA[:, k0 : k0 + 128],
                                    qA[:, k0 + lo : k0 + lo + n],
                                    start=True, stop=True)
                                nc.tensor.matmul(
                                    sc[:, 512 : 512 + n],
                                    qB[:, k0 : k0 + 128],
                                    qB[:, k0 + lo : k0 + lo + n],
                                    start=True, stop=True)
                                nc.scalar.activation(
                                    pt[:, :, lo : lo + n],
                                    sc[:].rearrange(
                                        "p (two n) -> p two n", two=2)[
                                        :, :, 0:n],
                                    EXP, scale=0.125)
                            # causal mask on the diagonal 128x128 block
                            nc.vector.tensor_mul(
                                pt[:, 0, 0:128], pt[:, 0, 0:128], MASK[:])
                            nc.vector.tensor_mul(
                                pt[:, 1, 0:128], pt[:, 1, 0:128], MASK[:])
                            pts.append(pt)
                        for hh in range(2):
                            h = 2 * jb + hh
                            for qc in range(4):
                                qs = qc * 512
                                nkb = 4 * qc + 4
                                C = ppv.tile([65, 512], F32, tag="pv")
                                for kb in range(nkb):
                                    po = max(0, kb * 128 - qs)
                                    ls = qs + po - kb * 128
                                    w = 512 - po
                                    nc.tensor.matmul(
                                        C[:, po:512],
                                        VA[:, kb, h, :],
                                        pts[kb][:, hh, ls : ls + w],
                                        start=(kb == 0),
                                        stop=(kb == nkb - 1))
                                # unnormalized ctxT evict (fp32r)
                                nc.vector.tensor_copy(
                                    CTXT[hh * 64 : hh * 64 + 64, jb,
                                         qs : qs + 512],
                                    C[0:64, :])
                                # stage denominator: row 64 -> DSTACK
                                i = h * 4 + qc
                                nc.vector.tensor_copy(
                                    DROW[64:65, :], C[64:65, :])
                                nc.sync.dma_start(
                                    out=DSTACK[4 * i : 4 * i + 4, :],
                                    in_=DROW[64:65, :])

                # ------------- normalization (post-attention) ---------
                with tc.tile_pool(name="pbc", bufs=2, space="PSUM") as pbc:
                    with nc.allow_low_precision(reason="f32r recip 1e-4 ok"):
                        nc.vector.reciprocal(RSTACK[:], DSTACK[:])
                    for i in range(32):
                        k, s = i % 4, i // 4
                        nc.sync.dma_start(
                            out=RROWS[32 * k : 32 * k + 1, s, :],
                            in_=RSTACK[4 * i : 4 * i + 4, :])
                    for i in range(32):
                        h, qc = i // 4, i % 4
                        hh, jb = h % 2, h // 2
                        k, s = i % 4, i // 4
                        BC = pbc.tile([128, 512], F32, tag="bc")
                        nc.tensor.matmul(
                            BC[:],
                            ONES_B[32 * k : 32 * k + 1, :],
                            RROWS[32 * k : 32 * k + 1, s, :],
                            start=True, stop=True,
                            tile_position=(32 * k, 0))
                        dst = CTXT[hh * 64 : hh * 64 + 64, jb,
                                   qc * 512 : qc * 512 + 512]
                        nc.vector.tensor_mul(dst, dst, BC[0:64, :])

                # ---------------- phase 3: out projection ----------------
                with tc.tile_pool(name="po", bufs=3) as po, \
                     tc.tile_pool(name="pop", bufs=4, space="PSUM") as pop:
                    WO = po.tile([128, 4, D], F32R, tag="wo")
                    for eb in range(4):
                        nc.sync.dma_start(
                            out=WO[:, eb, :],
                            in_=wo[eb * 128 : (eb + 1) * 128, :].bitcast(F32R))
                    for tb in range(16):
                        for nn in range(2):
                            ps = pop.tile([128, 512], F32, tag="pop")
                            for eb in range(4):
                                nc.tensor.matmul(
                                    ps[:],
                                    CTXT[:, eb, tb * 128 : (tb + 1) * 128],
                                    WO[:, eb, nn * 512 : (nn + 1) * 512],
                                    start=(eb == 0), stop=(eb == 3))
                            ob = po.tile([128, 512], F32, tag="ob")
                            nc.vector.tensor_copy(ob[:], ps[:])
                            nc.sync.dma_start(
                                out=outp[tb * 128 : (tb + 1) * 128,
                                         nn * 512 : (nn + 1) * 512],
                                in_=ob[:])
    nc.compile()
    return nc, {"xT": xT.name, "wq": wq.name, "wo": wo.name,
                "outp": outp.name}


def _get():
    if "nc" not in _CACHE:
        _CACHE["nc"], _CACHE["names"] = _build()
    return _CACHE["nc"], _CACHE["names"]


def _run(x, W_Q, W_out, trace=False):
    from concourse.bass_utils import run_bass_kernel_spmd

    nc, nm = _get()
    in_maps = []
    for c in range(N_CORES):
        b, hg = c // 2, c % 2
        in_maps.append({
            nm["xT"]: np.ascontiguousarray(x[b].T.astype(np.float32)),
            nm["wq"]: np.ascontiguousarray(
                W_Q[:, hg * EH : (hg + 1) * EH].astype(np.float32)),
            nm["wo"]: np.ascontiguousarray(
                W_out[hg * EH : (hg + 1) * EH, :].astype(np.float32)),
        })
    return run_bass_kernel_spmd(
        nc, in_maps, list(range(N_CORES)), trace=trace), nm


def kernel(x, W_Q, W_out, b_out):
    res, nm = _run(np.asarray(x), np.asarray(W_Q), np.asarray(W_out))
    bo = np.asarray(b_out, dtype=np.float32)
    out = np.empty((B, S, D), np.float32)
    for b in range(B):
        out[b] = (res.results[2 * b][nm["outp"]]
                  + res.results[2 * b + 1][nm["outp"]] + bo)
    return out


# revision 7
# speedup vs baseline: 1.6260x; 1.0254x over previous
"""Trainium2 Bass kernel for nn_MultiHeadAttention_39582418600023.

Model (reference bug preserved: Q = K = V = x @ W_Q):
  qkv = x @ W_Q; q,k,v = heads(qkv)
  out = softmax(causal(q k^T) / sqrt(dh)) v  ->  ctx @ W_out + b_out

Sharding (8 cores): data-parallel over batch (4) x tensor-parallel over
head groups (2).  Core c handles batch c//2, heads (c%2)*8 .. +8
(W_Q column-parallel, W_out row-parallel); host sums the two partial
out-projections per batch and adds the bias.

Per-core device kernel:
  dtypes: projections + out-proj in float32r (full PE rate at K=128,
  ~1e-4 rel err); attention matmuls in bf16 (score/prob quantization
  errors average out across the softmax dot products, ~1e-3 total).

  1. qkvT[e,t] (bf16, scores operands) and qkv->VA (bf16, ones-augmented
     V: VA[t,h,64]=1) via two fp32r projections from x^T.
  2. Head-pair loop (pairs share a 128-partition block of qkvT):
     scores ST[k,q] for both heads land in one [128,1024] PSUM tile via
     two concurrent row-tiled matmuls (base partitions 0/64); one
     exp-ACTIVATE per tile writes both heads' probs PT (bf16).
     Upper-tri 0/1 mask on the diagonal 128x128 block.
  3. PV: C[0:65] = sum_kb VA_kb^T @ PT_kb -> rows 0:64 unnormalized
     ctxT, row 64 softmax denominator.  ctxT evicted unnormalized
     (fp32r); denominators staged across 128 partitions.
  4. Post-attention: one reciprocal [128,128], denominator rows
     rebuilt at partitions {0,32,64,96} via DMA, broadcast to all
     partitions with K=1 matmuls, ctxT *= recip (DVE).
  5. out_partial = ctxT^T @ W_outc (fp32r); host reduces + bias.
"""
import os
import sys

sys.path.insert(0, "/opt/trn_rl_repo")
os.environ.setdefault("MYCRO_LOCAL_CACHE", "1")

import numpy as np

B, S, D = 4, 2048, 1024
NH, DH = 16, 64
EH = 512          # e-columns per core (8 local heads)
NHL = 8           # local heads
N_CORES = 8

_CACHE = {}


def _build():
    import concourse.mybir as mybir
    import concourse.tile as tile
    from concourse import bacc
    from concourse.masks import make_upper_triangular

    F32 = mybir.dt.float32
    F32R = mybir.dt.float32r
    BF16 = mybir.dt.bfloat16
    EXP = mybir.ActivationFunctionType.Exp

    nc = bacc.Bacc(None, target_bir_lowering=False, debug=True)
    with tile.TileContext(nc) as tc:
        with tc.tile_pool(name="dram", bufs=1, space="DRAM") as dram:
            xT = dram.tile([D, S], F32, kind="ExternalInput")      # x[b].T
            wq = dram.tile([D, EH], F32, kind="ExternalInput")     # W_Q cols
            wo = dram.tile([EH, D], F32, kind="ExternalInput")     # W_out rows
            outp = dram.tile([S, D], F32, kind="ExternalOutput")   # partial out

            with tc.tile_pool(name="persist", bufs=1) as pp:
                # qkvT: [e-block 128, eb, t], bf16 (scores operands)
                QKVT = pp.tile([128, 4, S], BF16)
                # ones-augmented V (bf16): [t%128, tb, h, 0:64]=V, [..,64]=1
                VA = pp.tile([128, 16, NHL, DH + 1], BF16)
                # unnormalized ctxT (bf16), same layout as QKVT
                CTXT = pp.tile([128, 4, S], BF16)
                IDN = pp.tile([128, 128], BF16)
                MASK = pp.tile([128, 128], F32)   # 1 on i<=j else 0
                make_upper_triangular(nc, MASK[:], val=1.0, diag=True)
                from concourse.masks import make_identity
                make_identity(nc, IDN[:])
                nc.vector.memset(VA[:, :, :, DH : DH + 1], 1.0)
                # denominator staging: (h,qc) pair i -> partitions 4i:4i+4
                DROW = pp.tile([65, 512], F32)
                DSTACK = pp.tile([128, 128], F32)
                RSTACK = pp.tile([128, 128], F32R)
                RROWS = pp.tile([128, 8, 512], F32R)
                ONES_F = pp.tile([128, 128], F32)
                ONES_B = pp.tile([128, 128], F32R)
                nc.vector.memset(ONES_F[:], 1.0)
                nc.vector.tensor_copy(ONES_B[:], ONES_F[:])

                # ---------------- phase 1: projections ----------------
                with tc.tile_pool(name="px", bufs=1) as px, \
                     tc.tile_pool(name="pj", bufs=4, space="PSUM") as pj:
                    XT = px.tile([128, 8, S], F32R)
                    WQ = px.tile([128, 8, EH], F32R)
                    for kc in range(8):
                        nc.sync.dma_start(
                            out=XT[:, kc, :],
                            in_=xT[kc * 128 : (kc + 1) * 128, :].bitcast(F32R))
                        nc.sync.dma_start(
                            out=WQ[:, kc, :],
                            in_=wq[kc * 128 : (kc + 1) * 128, :].bitcast(F32R))
                    # qkvT = wq^T @ xT  (evict on ACT -> bf16)
                    for eb in range(4):
                        for tn in range(4):
                            ps = pj.tile([128, 512], F32, tag="pj")
                            for kc in range(8):
                                nc.tensor.matmul(
                                    ps[:],
                                    WQ[:, kc, eb * 128 : (eb + 1) * 128],
                                    XT[:, kc, tn * 512 : (tn + 1) * 512],
                                    start=(kc == 0), stop=(kc == 7))
                            nc.vector.tensor_copy(
                                QKVT[:, eb, tn * 512 : (tn + 1) * 512], ps[:])
                    # V = transpose of qkvT 128x128 blocks -> VA (bf16)
                    with tc.tile_pool(name="ptr", bufs=4,
                                      space="PSUM") as ptr:
                        for jb in range(4):
                            for tb in range(16):
                                tp = ptr.tile([128, 128], BF16, tag="tp")
                                nc.tensor.transpose(
                                    tp[:],
                                    QKVT[:, jb, tb * 128 : (tb + 1) * 128],
                                    IDN[:])
                                nc.vector.tensor_copy(
                                    VA[:, tb, 2 * jb : 2 * jb + 2, 0:DH],
                                    tp[:].rearrange(
                                        "p (h d) -> p h d", h=2))

                # ---------------- phase 2: attention (head pairs) -----
                with tc.tile_pool(name="pt", bufs=1) as ptp, \
                     tc.tile_pool(name="psc", bufs=3, space="PSUM") as psc, \
                     tc.tile_pool(name="ppv", bufs=2, space="PSUM") as ppv:
                    for jb in range(4):          # head pair block
                        qA = QKVT[0:64, jb, :]   # head 2jb   [64, S]
                        qB = QKVT[64:128, jb, :]  # head 2jb+1
                        pts = []
                        for kb in range(16):
                            L = S - kb * 128
                            pt = ptp.tile([128, 2, L], BF16, tag=f"pt{kb}")
                            for lo in range(0, L, 512):
                                n = min(512, L - lo)
                                sc = psc.tile([128, 1024], F32, tag="sc")
                                k0 = kb * 128
                                nc.tensor.matmul(
                                    sc[:, 0:n],
                                    qA[:, k0 : k0 + 128],
                                    qA[:, k0 + lo : k0 + lo + n],
                                    start=True, stop=True)
                                nc.tensor.matmul(
                                    sc[:, 512 : 512 + n],
                                    qB[:, k0 : k0 + 128],
                                    qB[:, k0 + lo : k0 + lo + n],
                                    start=True, stop=True)
                                nc.scalar.activation(
                                    pt[:, :, lo : lo + n],
                                    sc[:].rearrange(
                                        "p (two n) -> p two n", two=2)[
                                        :, :, 0:n],
                                    EXP, scale=0.125)
                            # causal mask on the diagonal 128x128 block
                            nc.vector.tensor_mul(
                                pt[:, 0, 0:128], pt[:, 0, 0:128], MASK[:])
                            nc.vector.tensor_mul(
                                pt[:, 1, 0:128], pt[:, 1, 0:128], MASK[:])
                            pts.append(pt)
                        for hh in range(2):
                            h = 2 * jb + hh
                            for qc in range(4):
                                qs = qc * 512
                                nkb = 4 * qc + 4
                                C = ppv.tile([65, 512], F32, tag="pv")
                                for kb in range(nkb):
                                    po = max(0, kb * 128 - qs)
                                    ls = qs + po - kb * 128
                                    w = 512 - po
                                    nc.tensor.matmul(
                                        C[:, po:512],
                                        VA[:, kb, h, :],
                                        pts[kb][:, hh, ls : ls + w],
                                        start=(kb == 0),
                                        stop=(kb == nkb - 1))
                                # unnormalized ctxT evict (fp32r)
                                nc.vector.tensor_copy(
                                    CTXT[hh * 64 : hh * 64 + 64, jb,
                                         qs : qs + 512],
                                    C[0:64, :])
                                # stage denominator: row 64 -> DSTACK
                                i = h * 4 + qc
                                nc.vector.tensor_copy(
                                    DROW[64:65, :], C[64:65, :])
                                nc.sync.dma_start(
                                    out=DSTACK[4 * i : 4 * i + 4, :],
                                    in_=DROW[64:65, :])

                # ------------- normalization (post-attention) ---------
                with tc.tile_pool(name="pbc", bufs=2, space="PSUM") as pbc:
                    with nc.allow_low_precision(reason="f32r recip 1e-4 ok"):
                        nc.vector.reciprocal(RSTACK[:], DSTACK[:])
                    for i in range(32):
                        k, s = i % 4, i // 4
                        nc.sync.dma_start(
                            out=RROWS[32 * k : 32 * k + 1, s, :],
                            in_=RSTACK[4 * i : 4 * i + 4, :])
                    for i in range(32):
                        h, qc = i // 4, i % 4
                        hh, jb = h % 2, h // 2
                        k, s = i % 4, i // 4
                        BC = pbc.tile([128, 512], F32, tag="bc")
                        nc.tensor.matmul(
                            BC[:],
                            ONES_B[32 * k : 32 * k + 1, :],
                            RROWS[32 * k : 32 * k + 1, s, :],
                            start=True, stop=True,
                            tile_position=(32 * k, 0))
                        dst = CTXT[hh * 64 : hh * 64 + 64, jb,
                                   qc * 512 : qc * 512 + 512]
                        nc.vector.tensor_mul(dst, dst, BC[0:64, :])

                # ---------------- phase 3: out projection ----------------
                with tc.tile_pool(name="po", bufs=3) as po, \
                     tc.tile_pool(name="pop", bufs=4, space="PSUM") as pop:
                    WO = po.tile([128, 4, D], BF16, tag="wo")
                    WOF = po.tile([128, 4, D], F32, tag="wof")
                    for eb in range(4):
                        nc.sync.dma_start(
                            out=WOF[:, eb, :],
                            in_=wo[eb * 128 : (eb + 1) * 128, :])
                        nc.vector.tensor_copy(WO[:, eb, :], WOF[:, eb, :])
                    for tb in range(16):
                        for nn in range(2):
                            ps = pop.tile([128, 512], F32, tag="pop")
                            for eb in range(4):
                                nc.tensor.matmul(
                                    ps[:],
                                    CTXT[:, eb, tb * 128 : (tb + 1) * 128],
                                    WO[:, eb, nn * 512 : (nn + 1) * 512],
                                    start=(eb == 0), stop=(eb == 3))
                            ob = po.tile([128, 512], F32, tag="ob")
                            nc.vector.tensor_copy(ob[:], ps[:])
                            nc.sync.dma_start(
                                out=outp[tb * 128 : (tb + 1) * 128,
                                         nn * 512 : (nn + 1) * 512],
                                in_=ob[:])
    nc.compile()
    return nc, {"xT": xT.name, "wq": wq.name, "wo": wo.name,
                "outp": outp.name}


def _get():
    if "nc" not in _CACHE:
        _CACHE["nc"], _CACHE["names"] = _build()
    return _CACHE["nc"], _CACHE["names"]


def _run(x, W_Q, W_out, trace=False):
    from concourse.bass_utils import run_bass_kernel_spmd

    nc, nm = _get()
    in_maps = []
    for c in range(N_CORES):
        b, hg = c // 2, c % 2
        in_maps.append({
            nm["xT"]: np.ascontiguousarray(x[b].T.astype(np.float32)),
            nm["wq"]: np.ascontiguousarray(
                W_Q[:, hg * EH : (hg + 1) * EH].astype(np.float32)),
            nm["wo"]: np.ascontiguousarray(
                W_out[hg * EH : (hg + 1) * EH, :].astype(np.float32)),
        })
    return run_bass_kernel_spmd(
        nc, in_maps, list(range(N_CORES)), trace=trace), nm


def kernel(x, W_Q, W_out, b_out):
    res, nm = _run(np.asarray(x), np.asarray(W_Q), np.asarray(W_out))
    bo = np.asarray(b_out, dtype=np.float32)
    out = np.empty((B, S, D), np.float32)
    for b in range(B):
        out[b] = (res.results[2 * b][nm["outp"]]
                  + res.results[2 * b + 1][nm["outp"]] + bo)
    return out


# revision 8
# speedup vs baseline: 1.7446x; 1.0730x over previous
"""Trainium2 Bass kernel for nn_MultiHeadAttention_39582418600023.

Model (reference bug preserved: Q = K = V = x @ W_Q):
  qkv = x @ W_Q; q,k,v = heads(qkv)
  out = softmax(causal(q k^T) / sqrt(dh)) v  ->  ctx @ W_out + b_out

Sharding (8 cores): data-parallel over batch (4) x tensor-parallel over
head groups (2).  Core c handles batch c//2, heads (c%2)*8 .. +8
(W_Q column-parallel, W_out row-parallel); host sums the two partial
out-projections per batch and adds the bias.

Per-core device kernel:
  dtypes: projections + out-proj in float32r (full PE rate at K=128,
  ~1e-4 rel err); attention matmuls in bf16 (score/prob quantization
  errors average out across the softmax dot products, ~1e-3 total).

  1. qkvT[e,t] (bf16, scores operands) and qkv->VA (bf16, ones-augmented
     V: VA[t,h,64]=1) via two fp32r projections from x^T.
  2. Head-pair loop (pairs share a 128-partition block of qkvT):
     scores ST[k,q] for both heads land in one [128,1024] PSUM tile via
     two concurrent row-tiled matmuls (base partitions 0/64); one
     exp-ACTIVATE per tile writes both heads' probs PT (bf16).
     Upper-tri 0/1 mask on the diagonal 128x128 block.
  3. PV: C[0:65] = sum_kb VA_kb^T @ PT_kb -> rows 0:64 unnormalized
     ctxT, row 64 softmax denominator.  ctxT evicted unnormalized
     (fp32r); denominators staged across 128 partitions.
  4. Post-attention: one reciprocal [128,128], denominator rows
     rebuilt at partitions {0,32,64,96} via DMA, broadcast to all
     partitions with K=1 matmuls, ctxT *= recip (DVE).
  5. out_partial = ctxT^T @ W_outc (fp32r); host reduces + bias.
"""
import os
import sys

sys.path.insert(0, "/opt/trn_rl_repo")
os.environ.setdefault("MYCRO_LOCAL_CACHE", "1")

import numpy as np

B, S, D = 4, 2048, 1024
NH, DH = 16, 64
EH = 512          # e-columns per core (8 local heads)
NHL = 8           # local heads
N_CORES = 8

_CACHE = {}


def _build():
    import concourse.mybir as mybir
    import concourse.tile as tile
    from concourse import bacc
    from concourse.masks import make_upper_triangular

    F32 = mybir.dt.float32
    F32R = mybir.dt.float32r
    BF16 = mybir.dt.bfloat16
    EXP = mybir.ActivationFunctionType.Exp

    nc = bacc.Bacc(None, target_bir_lowering=False, debug=True)
    with tile.TileContext(nc) as tc:
        with tc.tile_pool(name="dram", bufs=1, space="DRAM") as dram:
            xT = dram.tile([D, S], F32, kind="ExternalInput")      # x[b].T
            wq = dram.tile([D, EH], F32, kind="ExternalInput")     # W_Q cols
            wo = dram.tile([EH, D], F32, kind="ExternalInput")     # W_out rows
            outp = dram.tile([S, D], F32, kind="ExternalOutput")   # partial out

            with tc.tile_pool(name="persist", bufs=1) as pp:
                # qkvT: [e-block 128, eb, t], bf16 (scores operands)
                QKVT = pp.tile([128, 4, S], BF16)
                # ones-augmented V (bf16): [t%128, tb, h, 0:64]=V, [..,64]=1
                VA = pp.tile([128, 16, NHL, DH + 1], BF16)
                # unnormalized ctxT (bf16), same layout as QKVT
                CTXT = pp.tile([128, 4, S], BF16)
                IDN = pp.tile([128, 128], BF16)
                MASK = pp.tile([128, 128], F32)   # 1 on i<=j else 0
                make_upper_triangular(nc, MASK[:], val=1.0, diag=True)
                from concourse.masks import make_identity
                make_identity(nc, IDN[:])
                nc.vector.memset(VA[:, :, :, DH : DH + 1], 1.0)
                # denominator staging: (h,qc) pair i -> partitions 4i:4i+4
                DROW = pp.tile([65, 512], F32)
                DSTACK = pp.tile([128, 128], F32)
                RSTACK = pp.tile([128, 128], F32R)
                RROWS = pp.tile([128, 8, 512], F32R)
                ONES_F = pp.tile([128, 128], F32)
                ONES_B = pp.tile([128, 128], F32R)
                nc.vector.memset(ONES_F[:], 1.0)
                nc.vector.tensor_copy(ONES_B[:], ONES_F[:])

                # ---------------- phase 1: projections ----------------
                with tc.tile_pool(name="px", bufs=1) as px, \
                     tc.tile_pool(name="pj", bufs=4, space="PSUM") as pj:
                    XT = px.tile([128, 8, S], F32R)
                    WQ = px.tile([128, 8, EH], F32R)
                    for kc in range(8):
                        nc.sync.dma_start(
                            out=XT[:, kc, :],
                            in_=xT[kc * 128 : (kc + 1) * 128, :].bitcast(F32R))
                        nc.sync.dma_start(
                            out=WQ[:, kc, :],
                            in_=wq[kc * 128 : (kc + 1) * 128, :].bitcast(F32R))
                    # qkvT = wq^T @ xT  (evict on ACT -> bf16)
                    for eb in range(4):
                        for tn in range(4):
                            ps = pj.tile([128, 512], F32, tag="pj")
                            for kc in range(8):
                                nc.tensor.matmul(
                                    ps[:],
                                    WQ[:, kc, eb * 128 : (eb + 1) * 128],
                                    XT[:, kc, tn * 512 : (tn + 1) * 512],
                                    start=(kc == 0), stop=(kc == 7))
                            nc.vector.tensor_copy(
                                QKVT[:, eb, tn * 512 : (tn + 1) * 512], ps[:])
                    # V = transpose of qkvT 128x128 blocks -> VA (bf16)
                    with tc.tile_pool(name="ptr", bufs=4,
                                      space="PSUM") as ptr:
                        for jb in range(4):
                            for tb in range(16):
                                tp = ptr.tile([128, 128], BF16, tag="tp")
                                nc.tensor.transpose(
                                    tp[:],
                                    QKVT[:, jb, tb * 128 : (tb + 1) * 128],
                                    IDN[:])
                                nc.vector.tensor_copy(
                                    VA[:, tb, 2 * jb : 2 * jb + 2, 0:DH],
                                    tp[:].rearrange(
                                        "p (h d) -> p h d", h=2))

                # ---------------- phase 2: attention (head pairs) -----
                # qc-outer / kb-inner: each PT chunk is consumed by PV
                # right after exp -> tiny PT pool, tight ACT pipeline.
                with tc.tile_pool(name="pt", bufs=4) as ptp, \
                     tc.tile_pool(name="psc", bufs=2, space="PSUM") as psc, \
                     tc.tile_pool(name="ppv", bufs=4, space="PSUM") as ppv:
                    for jb in range(4):          # head pair block
                        qA = QKVT[0:64, jb, :]   # head 2jb   [64, S]
                        qB = QKVT[64:128, jb, :]  # head 2jb+1
                        for qc in range(4):
                            qs = qc * 512
                            nkb = 4 * qc + 4
                            CA = ppv.tile([65, 512], F32, tag="pv")
                            CB = ppv.tile([65, 512], F32, tag="pv")
                            for kb in range(nkb):
                                k0 = kb * 128
                                q0 = max(k0, qs)
                                n = qs + 512 - q0
                                po = q0 - qs
                                sc = psc.tile([128, 1024], F32, tag="sc")
                                nc.tensor.matmul(
                                    sc[:, 0:n],
                                    qA[:, k0 : k0 + 128],
                                    qA[:, q0 : q0 + n],
                                    start=True, stop=True)
                                nc.tensor.matmul(
                                    sc[:, 512 : 512 + n],
                                    qB[:, k0 : k0 + 128],
                                    qB[:, q0 : q0 + n],
                                    start=True, stop=True)
                                pt = ptp.tile([128, 2, n], BF16, tag="pt")
                                nc.scalar.activation(
                                    pt[:],
                                    sc[:].rearrange(
                                        "p (two n) -> p two n", two=2)[
                                        :, :, 0:n],
                                    EXP, scale=0.125)
                                if k0 >= qs:   # diagonal 128x128 block
                                    nc.vector.tensor_mul(
                                        pt[:, 0, 0:128], pt[:, 0, 0:128],
                                        MASK[:])
                                    nc.vector.tensor_mul(
                                        pt[:, 1, 0:128], pt[:, 1, 0:128],
                                        MASK[:])
                                for hh, C in ((0, CA), (1, CB)):
                                    nc.tensor.matmul(
                                        C[:, po : po + n],
                                        VA[:, kb, 2 * jb + hh, :],
                                        pt[:, hh, :],
                                        start=(kb == 0),
                                        stop=(kb == nkb - 1))
                            for hh, C in ((0, CA), (1, CB)):
                                h = 2 * jb + hh
                                nc.vector.tensor_copy(
                                    CTXT[hh * 64 : hh * 64 + 64, jb,
                                         qs : qs + 512],
                                    C[0:64, :])
                                i = h * 4 + qc
                                nc.vector.tensor_copy(
                                    DROW[64:65, :], C[64:65, :])
                                nc.sync.dma_start(
                                    out=DSTACK[4 * i : 4 * i + 4, :],
                                    in_=DROW[64:65, :])

                # ------------- normalization (post-attention) ---------
                with tc.tile_pool(name="pbc", bufs=2, space="PSUM") as pbc:
                    with nc.allow_low_precision(reason="f32r recip 1e-4 ok"):
                        nc.vector.reciprocal(RSTACK[:], DSTACK[:])
                    for i in range(32):
                        k, s = i % 4, i // 4
                        nc.sync.dma_start(
                            out=RROWS[32 * k : 32 * k + 1, s, :],
                            in_=RSTACK[4 * i : 4 * i + 4, :])
                    for i in range(32):
                        h, qc = i // 4, i % 4
                        hh, jb = h % 2, h // 2
                        k, s = i % 4, i // 4
                        BC = pbc.tile([128, 512], F32, tag="bc")
                        nc.tensor.matmul(
                            BC[:],
                            ONES_B[32 * k : 32 * k + 1, :],
                            RROWS[32 * k : 32 * k + 1, s, :],
                            start=True, stop=True,
                            tile_position=(32 * k, 0))
                        dst = CTXT[hh * 64 : hh * 64 + 64, jb,
                                   qc * 512 : qc * 512 + 512]
                        nc.vector.tensor_mul(dst, dst, BC[0:64, :])

                # ---------------- phase 3: out projection ----------------
                with tc.tile_pool(name="po", bufs=3) as po, \
                     tc.tile_pool(name="pop", bufs=4, space="PSUM") as pop:
                    WO = po.tile([128, 4, D], BF16, tag="wo")
                    WOF = po.tile([128, 4, D], F32, tag="wof")
                    for eb in range(4):
                        nc.sync.dma_start(
                            out=WOF[:, eb, :],
                            in_=wo[eb * 128 : (eb + 1) * 128, :])
                        nc.vector.tensor_copy(WO[:, eb, :], WOF[:, eb, :])
                    for tb in range(16):
                        for nn in range(2):
                            ps = pop.tile([128, 512], F32, tag="pop")
                            for eb in range(4):
                                nc.tensor.matmul(
                                    ps[:],
                                    CTXT[:, eb, tb * 128 : (tb + 1) * 128],
                                    WO[:, eb, nn * 512 : (nn + 1) * 512],
                                    start=(eb == 0), stop=(eb == 3))
                            ob = po.tile([128, 512], F32, tag="ob")
                            nc.vector.tensor_copy(ob[:], ps[:])
                            nc.sync.dma_start(
                                out=outp[tb * 128 : (tb + 1) * 128,
                                         nn * 512 : (nn + 1) * 512],
                                in_=ob[:])
    nc.compile()
    return nc, {"xT": xT.name, "wq": wq.name, "wo": wo.name,
                "outp": outp.name}


def _get():
    if "nc" not in _CACHE:
        _CACHE["nc"], _CACHE["names"] = _build()
    return _CACHE["nc"], _CACHE["names"]


def _run(x, W_Q, W_out, trace=False):
    from concourse.bass_utils import run_bass_kernel_spmd

    nc, nm = _get()
    in_maps = []
    for c in range(N_CORES):
        b, hg = c // 2, c % 2
        in_maps.append({
            nm["xT"]: np.ascontiguousarray(x[b].T.astype(np.float32)),
            nm["wq"]: np.ascontiguousarray(
                W_Q[:, hg * EH : (hg + 1) * EH].astype(np.float32)),
            nm["wo"]: np.ascontiguousarray(
                W_out[hg * EH : (hg + 1) * EH, :].astype(np.float32)),
        })
    return run_bass_kernel_spmd(
        nc, in_maps, list(range(N_CORES)), trace=trace), nm


def kernel(x, W_Q, W_out, b_out):
    res, nm = _run(np.asarray(x), np.asarray(W_Q), np.asarray(W_out))
    bo = np.asarray(b_out, dtype=np.float32)
    out = np.empty((B, S, D), np.float32)
    for b in range(B):
        out[b] = (res.results[2 * b][nm["outp"]]
                  + res.results[2 * b + 1][nm["outp"]] + bo)
    return out
